# revision 1
# baseline (speedup 1.0000x reference)
"""ADDS loss kernel for Trainium2, 8 NeuronCores (SPMD, one class per core).

Math: for each class c, gather its (<=8) valid instances, transform the
class model points by pred/gt poses -> pp/gp point sets (8192 padded each,
invalid slots at a FAR location), then the 8192x8192 pairwise |pp-gp|^2
matrix with row mins (pred->gt) and column mins (gt->pred). Host finishes
with sqrt/means/masked sum.

Device per core (one class):
  - d2 computed directly by a K=15 bf16 matmul: contraction rows are the
    hi/lo bf16 split (3-term compensated product) of the 5 augmented
    coordinates (x, y, z, 1, p2) x (-2gx, -2gy, -2gz, g2, 1), so each PSUM
    tile holds exact-ish (~2^-17) d2 values.
  - ScalarE drains PSUM f32 -> bf16 SBUF; VectorE does all min work in
    bf16 2x perf mode (col-min running chain + row-min halving tree).
  - outputs: rowmin [128, 64] f32 and colrun [128, 16, 512] bf16; host
    reduces colrun over the partition axis and finishes sqrt/means/sum.
"""

import numpy as np
import ml_dtypes

import concourse.tile as tile
from concourse import bacc, mybir
from concourse.bass_utils import run_bass_kernel_spmd

F32 = mybir.dt.float32
BF16 = mybir.dt.bfloat16
AL = mybir.AluOpType

# Problem constants (hardcoded per harness contract)
B, N, C, P = 1, 64, 8, 1000
I = B * N            # 64 instances
M = I // C           # 8 instances per class (static cap, as in reference)
NPTS = M * P         # 8000 real points per side per class
MPAD = 8192          # padded point count (64 chunks x 128 = 16 blocks x 512)
NCHUNK = 64          # stationary chunks of 128
NBLK = 16            # moving blocks of 512
FAR = 1.0e6          # far-away location for invalid slots
EPS = 1e-12

_CACHED_NC = None


def _build_graph():
    """Per chunk of 128 pp points: 16 matmuls fill 4 PSUM groups; ScalarE
    drains each group to bf16 SBUF (the only other PSUM reader, freeing
    VectorE); VectorE runs all mins in bf16 2x perf mode: one 8192-wide
    col-min chain op per chunk plus a halving tree (8192->512) and a
    deferred per-4-chunk reduce for the row mins."""
    nc = bacc.Bacc()
    stat_d = nc.declare_dram_parameter("stat", [15, NCHUNK, 128], BF16, isOutput=False)
    mov_d = nc.declare_dram_parameter("mov", [15, NBLK, 512], BF16, isOutput=False)
    rowmin_d = nc.declare_dram_parameter("rowmin", [128, NCHUNK], F32, isOutput=True)
    colrun_d = nc.declare_dram_parameter("colrun", [128, NBLK, 512], BF16, isOutput=True)

    with tile.TileContext(nc) as tc:
        with (
            tc.tile_pool(name="consts", bufs=1) as consts,
            tc.tile_pool(name="psum", bufs=2, space="PSUM") as psum_pool,
            tc.tile_pool(name="grp", bufs=2) as grp_pool,
            tc.tile_pool(name="rbuf", bufs=2) as rbuf_pool,
        ):
            stat_sb = consts.tile([15, NCHUNK, 128], BF16)
            mov_sb = consts.tile([15, NBLK, 512], BF16)
            # split input DMAs so chunk-0 compute starts early
            nc.sync.dma_start(mov_sb[:, 0:2, :], mov_d[:, 0:2, :])
            nc.sync.dma_start(stat_sb[:, 0:2, :], stat_d[:, 0:2, :])
            nc.sync.dma_start(mov_sb[:, 2:8, :], mov_d[:, 2:8, :])
            nc.sync.dma_start(mov_sb[:, 8:, :], mov_d[:, 8:, :])
            nc.sync.dma_start(stat_sb[:, 2:16, :], stat_d[:, 2:16, :])
            nc.sync.dma_start(stat_sb[:, 16:, :], stat_d[:, 16:, :])

            colrun = consts.tile([128, NBLK, 512], BF16)
            rowmins = consts.tile([128, NCHUNK], F32)

            # chunk 63 is pure FAR padding: its row-mins are never read by the
            # host, and its column contributions (>=7e9) never win a col-min.
            NC_USED = NCHUNK - 1    # 63 = 31 pairs + 1 solo chunk
            NPAIR = NC_USED // 2

            def emit_chunk_mms(c, bb_slice):
                """16 matmuls for chunk c; ScalarE drains each 4-bank group
                into bb_slice ([128, 16, 512] bf16 view)."""
                for g in range(4):
                    pt = psum_pool.tile([128, 4, 512], F32, tag="pt")
                    for jj in range(4):
                        j = g * 4 + jj
                        nc.tensor.matmul(
                            pt[:, jj, :],
                            lhsT=stat_sb[:, c, :],
                            rhs=mov_sb[:, j, :],
                            start=True,
                            stop=True,
                        )
                    nc.scalar.copy(bb_slice[:, g * 4 : g * 4 + 4, :], pt[:])

            def emit_colmin(c, bb_slice):
                if c == 0:
                    # per-group copies: VectorE starts right after the first
                    # 4-bank drain instead of waiting for the whole chunk
                    for g in range(4):
                        nc.vector.tensor_copy(
                            colrun[:, g * 4 : g * 4 + 4, :],
                            bb_slice[:, g * 4 : g * 4 + 4, :],
                        )
                elif c == 1:
                    # pipeline fill: per-group mins keep VectorE fed while the
                    # first pair's drains are still completing
                    for g in range(4):
                        nc.vector.tensor_tensor(
                            colrun[:, g * 4 : g * 4 + 4, :],
                            bb_slice[:, g * 4 : g * 4 + 4, :],
                            colrun[:, g * 4 : g * 4 + 4, :],
                            AL.min,
                        )
                elif c == NC_USED - 1:
                    # split halves so each colrun half DMAs out while the
                    # other half is still being min'd
                    for hh in range(2):
                        nc.vector.tensor_tensor(
                            colrun[:, hh * 8 : hh * 8 + 8, :],
                            bb_slice[:, hh * 8 : hh * 8 + 8, :],
                            colrun[:, hh * 8 : hh * 8 + 8, :],
                            AL.min,
                        )
                        nc.sync.dma_start(
                            colrun_d[:, hh * 8 : hh * 8 + 8, :],
                            colrun[:, hh * 8 : hh * 8 + 8, :],
                        )
                else:
                    nc.vector.tensor_tensor(colrun[:], bb_slice[:], colrun[:], AL.min)

            rbatch = rbuf_pool.tile([128, 8, 512], BF16, tag="rbatch")
            flushed = 0

            def flush_rowmins(hi_slot):
                nonlocal rbatch, flushed
                nc.vector.tensor_reduce(
                    rowmins[:, flushed : flushed + hi_slot],
                    rbatch[:, 0:hi_slot, :],
                    mybir.AxisListType.X,
                    AL.min,
                )
                flushed += hi_slot
                rbatch = rbuf_pool.tile([128, 8, 512], BF16, tag="rbatch")

            for p in range(NPAIR):
                # two chunks share one row-min tree (3D APs, halved op count)
                bbp = grp_pool.tile([128, 2, NBLK, 512], BF16, tag="bbp")
                for ci in range(2):
                    c = 2 * p + ci
                    emit_chunk_mms(c, bbp[:, ci])
                    emit_colmin(c, bbp[:, ci])
                bv = bbp.rearrange("p a b c -> p a (b c)")
                t = grp_pool.tile([128, 2, 4096], BF16, tag="t")
                nc.vector.tensor_tensor(
                    t[:], bv[:, :, 0:4096], bv[:, :, 4096:8192], AL.min
                )
                r = grp_pool.tile([128, 2, 2048], BF16, tag="r")
                nc.vector.tensor_tensor(
                    r[:], t[:, :, 0:2048], t[:, :, 2048:4096], AL.min
                )
                half = grp_pool.tile([128, 2, 1024], BF16, tag="half")
                nc.vector.tensor_tensor(
                    half[:], r[:, :, 0:1024], r[:, :, 1024:2048], AL.min
                )
                slot = 2 * (p % 4)
                nc.vector.tensor_tensor(
                    rbatch[:, slot : slot + 2, :],
                    half[:, :, 0:512],
                    half[:, :, 512:1024],
                    AL.min,
                )
                if p % 4 == 3:
                    flush_rowmins(8)

            # solo tail chunk (62) — reuse the pair-sized buffers, half used
            c = NC_USED - 1
            bbp = grp_pool.tile([128, 2, NBLK, 512], BF16, tag="bbp")
            emit_chunk_mms(c, bbp[:, 0])
            emit_colmin(c, bbp[:, 0])
            bvs = bbp[:, 0].rearrange("p a b -> p (a b)")
            t = grp_pool.tile([128, 2, 4096], BF16, tag="t")
            nc.vector.tensor_tensor(
                t[:, 0, :], bvs[:, 0:4096], bvs[:, 4096:8192], AL.min
            )
            r = grp_pool.tile([128, 2, 2048], BF16, tag="r")
            nc.vector.tensor_tensor(
                r[:, 0, :], t[:, 0, 0:2048], t[:, 0, 2048:4096], AL.min
            )
            half = grp_pool.tile([128, 2, 1024], BF16, tag="half")
            nc.vector.tensor_tensor(
                half[:, 0, :], r[:, 0, 0:1024], r[:, 0, 1024:2048], AL.min
            )
            tail_slot = 2 * (NPAIR % 4)
            nc.vector.tensor_tensor(
                rbatch[:, tail_slot : tail_slot + 1, :],
                half[:, 0, 0:512],
                half[:, 0, 512:1024],
                AL.min,
            )
            flush_rowmins(tail_slot + 1)

            nc.sync.dma_start(rowmin_d[:, 0:NC_USED], rowmins[:, 0:NC_USED])
    nc.compile()
    return nc


def _split_bf16(x):
    """Return (hi, lo) bf16 arrays with hi + lo ~= x (f32)."""
    x = x.astype(np.float32)
    hi = x.astype(ml_dtypes.bfloat16)
    lo = (x - hi.astype(np.float32)).astype(ml_dtypes.bfloat16)
    return hi, lo


def _pack_class(pp, gp):
    """pp, gp: [MPAD, 3] f32 -> stat [15, NCHUNK, 128] bf16, mov [15, NBLK, 512] bf16."""
    p2 = (pp.astype(np.float32) ** 2).sum(1)
    g2 = (gp.astype(np.float32) ** 2).sum(1)
    stat5 = np.empty((5, MPAD), np.float32)
    stat5[0:3] = pp.T
    stat5[3] = 1.0
    stat5[4] = p2
    mov5 = np.empty((5, MPAD), np.float32)
    mov5[0:3] = -2.0 * gp.T
    mov5[3] = g2
    mov5[4] = 1.0
    s_hi, s_lo = _split_bf16(stat5)
    m_hi, m_lo = _split_bf16(mov5)
    # 3-term compensated product: hi*hi + hi*lo + lo*hi
    stat = np.concatenate([s_hi, s_hi, s_lo], axis=0)  # [15, MPAD]
    mov = np.concatenate([m_hi, m_lo, m_hi], axis=0)   # [15, MPAD]
    return (
        stat.reshape(15, NCHUNK, 128),
        mov.reshape(15, NBLK, 512),
    )


def kernel(pred_rot_matrix, pred_trans, target_rot_matrix, target_trans,
           model_points, fg_mask, class_ids):
    global _CACHED_NC
    predR = np.asarray(pred_rot_matrix, np.float32).reshape(I, 3, 3)
    predt = np.asarray(pred_trans, np.float32).reshape(I, 3)
    gtR = np.asarray(target_rot_matrix, np.float32).reshape(I, 3, 3)
    gtt = np.asarray(target_trans, np.float32).reshape(I, 3)
    pts = np.asarray(model_points, np.float32)  # [C, P, 3]
    fg = np.asarray(fg_mask).reshape(I).astype(bool)
    cls = np.asarray(class_ids).reshape(I).astype(np.int64)

    in_maps = []
    valid_counts = []
    for c in range(C):
        m = fg & (cls == c)
        idx = np.argsort(~m, kind="stable")[:M]
        valid = m[idx]
        k = int(valid.sum())
        valid_counts.append(k)
        pp = np.full((MPAD, 3), FAR, np.float32)
        gp = np.full((MPAD, 3), FAR, np.float32)
        for slot in range(k):
            i = idx[slot]
            xc = pts[cls[i]]  # the instance's own class points (== c for valid)
            pp[slot * P : (slot + 1) * P] = xc @ predR[i].T + predt[i]
            gp[slot * P : (slot + 1) * P] = xc @ gtR[i].T + gtt[i]
        stat, mov = _pack_class(pp, gp)
        in_maps.append({"stat": stat, "mov": mov})

    if _CACHED_NC is None:
        _CACHED_NC = _build_graph()
    res = run_bass_kernel_spmd(_CACHED_NC, in_maps, core_ids=list(range(8)))

    total = np.float32(0.0)
    for c in range(C):
        k = valid_counts[c]
        if k == 0:
            continue
        r = res.results[c]
        rowmin = np.asarray(r["rowmin"], np.float32).T.reshape(-1)  # [MPAD], m = c*128+p
        colmin = np.asarray(r["colrun"], np.float32).min(axis=0).reshape(-1)  # [MPAD]
        d_p2g = np.sqrt(np.maximum(rowmin[: k * P], EPS)).reshape(k, P).mean(axis=1)
        d_g2p = np.sqrt(np.maximum(colmin[: k * P], EPS)).reshape(k, P).mean(axis=1)
        total += (0.5 * (d_p2g + d_g2p)).sum()

    n_fg = int(fg.sum())
    if n_fg > 0:
        out = np.float32(total / np.float32(max(n_fg, 1)))
    else:
        out = np.float32(0.0)
    return np.asarray(out, dtype=np.float32)



# revision 13
# speedup vs baseline: 6.3179x; 6.3179x over previous
"""ADDS loss kernel for Trainium2, 8 NeuronCores (SPMD, one class per core).

Math: for each class c, gather its (<=8) valid instances, transform the
class model points by pred/gt poses -> pp/gp point sets.  The loss needs,
for every point, the distance to the nearest point of the other set
(both directions), then sqrt/means/masked sum.

Instead of the full 8192x8192 pairwise matrix (vector-engine bound at
~600us), the host runs an IVF-style coarse selection: candidates are
kd-split into 2^LDEPTH balanced leaves, queries into 64 blocks of 128.
Per query block the host ranks leaves by the exact-safe bound
min_q(lowerbound(q,leaf) - upperbound(q)) and keeps the top LEAVES_TAKE
leaves (CAND candidate points).  The device then computes the exact
d2 = |p|^2+|g|^2-2p.g for the 128x CAND matrix of every block (K=15
compensated bf16 matmul, ~2^-17 accurate) and min-reduces each row:

  - per 4-block group: 4 matmuls fill 4 PSUM banks.
  - first touch of PSUM is split between ScalarE (bf16 drain, then a
    VectorE pair-min) and VectorE (direct pair-min of the two PSUM
    halves) to balance the two engines.
  - a bf16 halving tree reduces to 32 wide; a deferred batched tree
    finishes 32 -> 1 across all 128 block-dirs at once.

Host finishes with sqrt/means/masked sum (identical to reference tail).
"""

import numpy as np
import ml_dtypes

import concourse.tile as tile
from concourse import bacc, mybir
from concourse.bass_utils import run_bass_kernel_spmd

F32 = mybir.dt.float32
BF16 = mybir.dt.bfloat16
AL = mybir.AluOpType

# Problem constants (hardcoded per harness contract)
B, N, C, P = 1, 64, 8, 1000
I = B * N            # 64 instances
M = I // C           # 8 instances per class (static cap, as in reference)
NPAD = 8192          # padded point count per side per class
QBLK = 64            # query blocks of 128 per direction
NBD = 2 * QBLK       # block-dirs per core (2 directions)
NGRP = NBD // 4      # device groups of 4 block-dirs
EPS = 1e-12

# IVF selection config
LDEPTH = 10                      # 2^10 = 1024 leaves of 8 points
LSZ = NPAD >> LDEPTH             # leaf size (8)
CAND = 512                       # candidate points per query block
LEAVES_TAKE = CAND // LSZ        # leaves kept per block
# per-group first-touch schedule: True = split drain (VectorE reads PSUM
# for one half), False = ScalarE drains all of PSUM. ~0.4 split fraction
# balances the two engines.
SPLIT_PATTERN = (True, False, True, False, False)

_CACHED_NC = None


def _build_graph():
    nc = bacc.Bacc()
    stat_d = nc.declare_dram_parameter("stat", [15, NBD, 128], BF16, isOutput=False)
    mov_d = nc.declare_dram_parameter("mov", [15, NBD, CAND], BF16, isOutput=False)
    rowmin_d = nc.declare_dram_parameter("rowmin", [128, NBD, 1], F32, isOutput=True)

    with tile.TileContext(nc) as tc:
        with (
            tc.tile_pool(name="consts", bufs=1) as consts,
            tc.tile_pool(name="psum", bufs=2, space="PSUM") as psum_pool,
            tc.tile_pool(name="grp", bufs=3) as grp_pool,
            tc.tile_pool(name="fin", bufs=1) as fin_pool,
        ):
            stat_sb = consts.tile([15, NBD, 128], BF16)
            mov_sb = consts.tile([15, NBD, CAND], BF16)
            # split input DMAs so group-0 compute starts early
            nc.sync.dma_start(mov_sb[:, 0:4, :], mov_d[:, 0:4, :])
            nc.sync.dma_start(stat_sb[:, 0:4, :], stat_d[:, 0:4, :])
            nc.sync.dma_start(mov_sb[:, 4:16, :], mov_d[:, 4:16, :])
            nc.sync.dma_start(stat_sb[:, 4:16, :], stat_d[:, 4:16, :])
            nc.sync.dma_start(mov_sb[:, 16:48, :], mov_d[:, 16:48, :])
            nc.sync.dma_start(stat_sb[:, 16:, :], stat_d[:, 16:, :])
            nc.sync.dma_start(mov_sb[:, 48:88, :], mov_d[:, 48:88, :])
            nc.sync.dma_start(mov_sb[:, 88:, :], mov_d[:, 88:, :])

            collect = consts.tile([128, NBD, 32], BF16)
            rowmins = consts.tile([128, NBD, 1], F32)

            HC = CAND // 2
            for g in range(NGRP):
                pt = psum_pool.tile([128, 4, CAND], F32, tag="pt")
                for j in range(4):
                    bd = 4 * g + j
                    nc.tensor.matmul(
                        pt[:, j, :],
                        lhsT=stat_sb[:, bd, :],
                        rhs=mov_sb[:, bd, :],
                        start=True,
                        stop=True,
                    )
                bb = grp_pool.tile([128, 4, HC], BF16, tag="bb")
                if SPLIT_PATTERN[g % len(SPLIT_PATTERN)]:
                    # type-B: ScalarE drains one half; VectorE fuses the
                    # pair-min into its own PSUM drain of the other half
                    sc = grp_pool.tile([128, 4, HC], BF16, tag="sch")
                    nc.scalar.copy(sc[:], pt[:, :, 0:HC])
                    nc.vector.tensor_tensor(
                        bb[:], pt[:, :, HC:CAND], sc[:], AL.min
                    )
                else:
                    # type-A: ScalarE drains PSUM -> bf16; VectorE pair-min 2x
                    sc = grp_pool.tile([128, 4, CAND], BF16, tag="sc")
                    nc.scalar.copy(sc[:], pt[:])
                    nc.vector.tensor_tensor(
                        bb[:], sc[:, :, 0:HC], sc[:, :, HC:CAND], AL.min
                    )
                # bf16 halving tree down to 32 wide
                cur = bb
                w = HC
                while w > 64:
                    nxt = grp_pool.tile([128, 4, w // 2], BF16, tag=f"t{w}")
                    nc.vector.tensor_tensor(
                        nxt[:], cur[:, :, 0 : w // 2], cur[:, :, w // 2 : w], AL.min
                    )
                    cur = nxt
                    w //= 2
                nc.vector.tensor_tensor(
                    collect[:, 4 * g : 4 * g + 4, :],
                    cur[:, :, 0:32],
                    cur[:, :, 32:64],
                    AL.min,
                )

            # deferred batched final tree: [128, NBD, 32] -> [128, NBD]
            f16 = fin_pool.tile([128, NBD, 16], BF16, tag="f16")
            nc.vector.tensor_tensor(
                f16[:], collect[:, :, 0:16], collect[:, :, 16:32], AL.min
            )
            f8 = fin_pool.tile([128, NBD, 8], BF16, tag="f8")
            nc.vector.tensor_tensor(f8[:], f16[:, :, 0:8], f16[:, :, 8:16], AL.min)
            f4 = fin_pool.tile([128, NBD, 4], BF16, tag="f4")
            nc.vector.tensor_tensor(f4[:], f8[:, :, 0:4], f8[:, :, 4:8], AL.min)
            f2 = fin_pool.tile([128, NBD, 2], BF16, tag="f2")
            nc.vector.tensor_tensor(f2[:], f4[:, :, 0:2], f4[:, :, 2:4], AL.min)
            nc.vector.tensor_tensor(
                rowmins[:],
                f2[:, :, 0:1],
                f2[:, :, 1:2],
                AL.min,
            )
            nc.sync.dma_start(rowmin_d[:], rowmins[:])
    nc.compile()
    return nc


def _split_bf16(x):
    """Return (hi, lo) bf16 arrays with hi + lo ~= x (f32)."""
    x = x.astype(np.float32)
    hi = x.astype(ml_dtypes.bfloat16)
    lo = (x - hi.astype(np.float32)).astype(ml_dtypes.bfloat16)
    return hi, lo


def _aug5(pts, side):
    """pts [..., 3] -> aug [..., 5] rows (x,y,z,1,p2) or (-2x,-2y,-2z,g2,1)."""
    sq = (pts.astype(np.float32) ** 2).sum(-1)
    out = np.empty(pts.shape[:-1] + (5,), np.float32)
    if side == "stat":
        out[..., 0:3] = pts
        out[..., 3] = 1.0
        out[..., 4] = sq
    else:
        out[..., 0:3] = -2.0 * pts
        out[..., 3] = sq
        out[..., 4] = 1.0
    return out


def _comp15(aug, stationary):
    """aug [..., 5] f32 -> compensated bf16 [..., 15] (3-term product split)."""
    hi, lo = _split_bf16(aug)
    if stationary:
        return np.concatenate([hi, hi, lo], axis=-1)
    return np.concatenate([hi, lo, hi], axis=-1)


def _pad_dup(X):
    idx = np.concatenate([np.arange(len(X)), np.arange(NPAD - len(X))])
    return X[idx], idx


def _kd_order(X, depth):
    """Balanced kd ordering: permutation putting X into 2^depth equal leaves."""
    n = len(X)
    perm = np.arange(n)[None, :]           # [nseg, seglen]
    for _ in range(depth):
        seg = X[perm]                      # [nseg, seglen, 3]
        ax = np.argmax(seg.max(1) - seg.min(1), axis=1)        # [nseg]
        vals = np.take_along_axis(seg, ax[:, None, None], 2)[:, :, 0]
        order = np.argsort(vals, axis=1, kind="stable")
        perm = np.take_along_axis(perm, order, 1)
        perm = perm.reshape(perm.shape[0] * 2, perm.shape[1] // 2)
    return perm.reshape(-1)


def _select_blocks(Q, X):
    """IVF selection for one direction of one class.

    Q: [nq, 3] queries, X: [nx, 3] candidates (nq, nx >= 1).
    Returns (qord [NPAD], stat15 [QBLK,128,15], mov15 [QBLK,CAND,15])."""
    Qp, _ = _pad_dup(Q)
    Xp, _ = _pad_dup(X)
    qord = _kd_order(Qp, 6)
    xord = _kd_order(Xp, LDEPTH)
    Xo = Xp[xord].reshape(-1, LSZ, 3)                  # [NL, LSZ, 3]
    cent = Xo.mean(1)
    dif = Xo - cent[:, None, :]
    dist_c = np.sqrt((dif * dif).sum(2))
    rad = dist_c.max(1)
    rep = Xo[np.arange(len(Xo)), dist_c.argmin(1)]

    Qs = Qp[qord]
    q2 = (Qs * Qs).sum(1)[:, None]
    dc = np.sqrt(np.maximum(q2 + (cent * cent).sum(1)[None, :] - 2.0 * Qs @ cent.T, 0))
    drep = np.sqrt(np.maximum(q2 + (rep * rep).sum(1)[None, :] - 2.0 * Qs @ rep.T, 0))
    ub = drep.min(1)
    score = (dc - rad[None, :]) - ub[:, None]
    score_b = score.reshape(QBLK, 128, -1).min(1)      # [QBLK, NL]
    take = np.argpartition(score_b, LEAVES_TAKE - 1, axis=1)[:, :LEAVES_TAKE]

    cand = Xo[take].reshape(QBLK, CAND, 3)             # [QBLK, CAND, 3]
    stat15 = _comp15(_aug5(Qs.reshape(QBLK, 128, 3), "stat"), True)
    mov15 = _comp15(_aug5(cand, "mov"), False)
    return qord, stat15, mov15


def kernel(pred_rot_matrix, pred_trans, target_rot_matrix, target_trans,
           model_points, fg_mask, class_ids):
    global _CACHED_NC
    predR = np.asarray(pred_rot_matrix, np.float32).reshape(I, 3, 3)
    predt = np.asarray(pred_trans, np.float32).reshape(I, 3)
    gtR = np.asarray(target_rot_matrix, np.float32).reshape(I, 3, 3)
    gtt = np.asarray(target_trans, np.float32).reshape(I, 3)
    pts = np.asarray(model_points, np.float32)  # [C, P, 3]
    fg = np.asarray(fg_mask).reshape(I).astype(bool)
    cls = np.asarray(class_ids).reshape(I).astype(np.int64)

    in_maps = []
    meta = []
    for c in range(C):
        m = fg & (cls == c)
        idx = np.argsort(~m, kind="stable")[:M]
        valid = m[idx]
        k = int(valid.sum())
        if k == 0:
            meta.append(None)
            in_maps.append({
                "stat": np.zeros((15, NBD, 128), ml_dtypes.bfloat16),
                "mov": np.zeros((15, NBD, CAND), ml_dtypes.bfloat16),
            })
            continue
        pp = np.concatenate(
            [pts[cls[i]] @ predR[i].T + predt[i] for i in idx[:k]], 0
        ).astype(np.float32)
        gp = np.concatenate(
            [pts[cls[i]] @ gtR[i].T + gtt[i] for i in idx[:k]], 0
        ).astype(np.float32)
        qord0, stat0, mov0 = _select_blocks(pp, gp)
        qord1, stat1, mov1 = _select_blocks(gp, pp)
        # stat: [15, NBD, 128]; mov: [15, NBD, CAND]
        stat = np.concatenate([stat0, stat1], 0).transpose(2, 0, 1)
        mov = np.concatenate([mov0, mov1], 0).transpose(2, 0, 1)
        meta.append((k, qord0, qord1))
        in_maps.append({"stat": np.ascontiguousarray(stat),
                        "mov": np.ascontiguousarray(mov)})

    if _CACHED_NC is None:
        _CACHED_NC = _build_graph()
    res = run_bass_kernel_spmd(_CACHED_NC, in_maps, core_ids=list(range(8)))

    total = np.float32(0.0)
    for c in range(C):
        if meta[c] is None:
            continue
        k, qord0, qord1 = meta[c]
        rm = np.asarray(res.results[c]["rowmin"], np.float32).reshape(128, NBD)
        d_acc = np.zeros(k, np.float64)
        for d, qord in ((0, qord0), (1, qord1)):
            vals = rm[:, d * QBLK : (d + 1) * QBLK].T.reshape(-1)  # sorted order
            dmin = np.empty(NPAD, np.float32)
            dmin[qord] = vals
            dd = np.sqrt(np.maximum(dmin[: k * P], EPS))
            d_acc += dd.reshape(k, P).mean(1)
        total += np.float32((0.5 * d_acc).sum())

    n_fg = int(fg.sum())
    if n_fg > 0:
        out = np.float32(total / np.float32(max(n_fg, 1)))
    else:
        out = np.float32(0.0)
    return np.asarray(out, dtype=np.float32)


# revision 15
# speedup vs baseline: 9.9601x; 1.5765x over previous
"""ADDS loss kernel for Trainium2, 8 NeuronCores (SPMD, one class per core).

Math: for each class c, gather its (<=8) valid instances, transform the
class model points by pred/gt poses -> pp/gp point sets.  The loss needs,
for every point, the distance to the nearest point of the other set
(both directions), then sqrt/means/masked sum.

Instead of the full 8192x8192 pairwise matrix (vector-engine bound at
~600us), the host runs an IVF-style coarse selection: candidates are
kd-split into 2^LDEPTH balanced leaves, queries into 64 blocks of 128.
Per query block the host ranks leaves by the exact-safe bound
min_q(lowerbound(q,leaf) - upperbound(q)) and keeps the top LEAVES_TAKE
leaves (CAND candidate points).  The device then computes the exact
d2 = |p|^2+|g|^2-2p.g for the 128x CAND matrix of every block (K=15
compensated bf16 matmul, ~2^-17 accurate) and min-reduces each row:

  - per 4-block group: 4 matmuls fill 4 PSUM banks.
  - first touch of PSUM is split between ScalarE (bf16 drain, then a
    VectorE pair-min) and VectorE (direct pair-min of the two PSUM
    halves) to balance the two engines.
  - a bf16 halving tree reduces to 32 wide; a deferred batched tree
    finishes 32 -> 1 across all 128 block-dirs at once.

Host finishes with sqrt/means/masked sum (identical to reference tail).
"""

import numpy as np
import ml_dtypes

import concourse.tile as tile
from concourse import bacc, mybir
from concourse.bass_utils import run_bass_kernel_spmd

F32 = mybir.dt.float32
BF16 = mybir.dt.bfloat16
AL = mybir.AluOpType

# Problem constants (hardcoded per harness contract)
B, N, C, P = 1, 64, 8, 1000
I = B * N            # 64 instances
M = I // C           # 8 instances per class (static cap, as in reference)
NPAD = 8192          # padded point count per side per class
QBLK = 64            # query blocks of 128 per direction
NBD = 2 * QBLK       # block-dirs per core (2 directions)
NGRP = NBD // 4      # device groups of 4 block-dirs
EPS = 1e-12

# IVF selection config
LDEPTH = 11                      # 2^11 = 2048 leaves of 4 points
LSZ = NPAD >> LDEPTH             # leaf size (4)
CAND = 256                       # candidate points per query block
LEAVES_TAKE = CAND // LSZ        # leaves kept per block
# per-group first-touch schedule: True = split drain (VectorE reads PSUM
# for one half), False = ScalarE drains all of PSUM. ~0.5 split fraction
# balances the two engines.
SPLIT_PATTERN = (True, False)

_CACHED_NC = None


def _build_graph():
    nc = bacc.Bacc()
    stat_d = nc.declare_dram_parameter("stat", [15, NBD, 128], BF16, isOutput=False)
    mov_d = nc.declare_dram_parameter("mov", [15, NBD, CAND], BF16, isOutput=False)
    rowmin_d = nc.declare_dram_parameter("rowmin", [128, NBD, 1], F32, isOutput=True)

    HC = CAND // 2

    with tile.TileContext(nc) as tc:
        with (
            tc.tile_pool(name="consts", bufs=1) as consts,
            tc.tile_pool(name="psum", bufs=3, space="PSUM") as psum_pool,
            tc.tile_pool(name="mov", bufs=6) as mov_pool,
            tc.tile_pool(name="grp", bufs=3) as grp_pool,
            tc.tile_pool(name="fin", bufs=1) as fin_pool,
        ):
            stat_sb = consts.tile([15, NBD, 128], BF16)
            nc.sync.dma_start(stat_sb[:, 0:16, :], stat_d[:, 0:16, :])
            nc.sync.dma_start(stat_sb[:, 16:, :], stat_d[:, 16:, :])

            collect = consts.tile([128, NBD, HC], BF16)
            rowmins = consts.tile([128, NBD, 1], F32)

            def final_tree(lo, hi):
                """Reduce collect[:, lo:hi, :HC] -> rowmins[:, lo:hi]."""
                nbd = hi - lo
                cur = collect[:, lo:hi, :]
                w = HC
                while w > 1:
                    h = w // 2
                    if h == 1:
                        nxt = rowmins[:, lo:hi, :]
                    else:
                        nxt = fin_pool.tile([128, NBD // 2, h], BF16, tag=f"f{h}")
                        nxt = nxt[:, 0:nbd, :]
                    nc.vector.tensor_tensor(
                        nxt[:], cur[:, :, 0:h], cur[:, :, h:w], AL.min
                    )
                    cur = nxt
                    w = h

            for g in range(NGRP):
                mt = mov_pool.tile([15, 4, CAND], BF16, tag="mt")
                nc.sync.dma_start(mt[:], mov_d[:, 4 * g : 4 * g + 4, :])
                pt = psum_pool.tile([128, 4, CAND], F32, tag="pt")
                for j in range(4):
                    nc.tensor.matmul(
                        pt[:, j, :],
                        lhsT=stat_sb[:, 4 * g + j, :],
                        rhs=mt[:, j, :],
                        start=True,
                        stop=True,
                    )
                out = collect[:, 4 * g : 4 * g + 4, :]
                if SPLIT_PATTERN[g % len(SPLIT_PATTERN)]:
                    # type-B: ScalarE drains one half; VectorE fuses the
                    # pair-min into its own PSUM drain of the other half
                    sc = grp_pool.tile([128, 4, HC], BF16, tag="sch")
                    nc.scalar.copy(sc[:], pt[:, :, 0:HC])
                    nc.vector.tensor_tensor(
                        out[:], pt[:, :, HC:CAND], sc[:], AL.min
                    )
                else:
                    # type-A: ScalarE drains PSUM -> bf16; VectorE pair-min 2x
                    sc = grp_pool.tile([128, 4, CAND], BF16, tag="sc")
                    nc.scalar.copy(sc[:], pt[:])
                    nc.vector.tensor_tensor(
                        out[:], sc[:, :, 0:HC], sc[:, :, HC:CAND], AL.min
                    )
                if g == NGRP // 2 - 1:
                    final_tree(0, NBD // 2)
            final_tree(NBD // 2, NBD)
            nc.sync.dma_start(rowmin_d[:, 0 : NBD // 2, :],
                              rowmins[:, 0 : NBD // 2, :])
            nc.sync.dma_start(rowmin_d[:, NBD // 2 :, :],
                              rowmins[:, NBD // 2 :, :])
    nc.compile()
    return nc


def _split_bf16(x):
    """Return (hi, lo) bf16 arrays with hi + lo ~= x (f32)."""
    x = x.astype(np.float32)
    hi = x.astype(ml_dtypes.bfloat16)
    lo = (x - hi.astype(np.float32)).astype(ml_dtypes.bfloat16)
    return hi, lo


def _aug5(pts, side):
    """pts [..., 3] -> aug [..., 5] rows (x,y,z,1,p2) or (-2x,-2y,-2z,g2,1)."""
    sq = (pts.astype(np.float32) ** 2).sum(-1)
    out = np.empty(pts.shape[:-1] + (5,), np.float32)
    if side == "stat":
        out[..., 0:3] = pts
        out[..., 3] = 1.0
        out[..., 4] = sq
    else:
        out[..., 0:3] = -2.0 * pts
        out[..., 3] = sq
        out[..., 4] = 1.0
    return out


def _comp15(aug, stationary):
    """aug [..., 5] f32 -> compensated bf16 [..., 15] (3-term product split)."""
    hi, lo = _split_bf16(aug)
    if stationary:
        return np.concatenate([hi, hi, lo], axis=-1)
    return np.concatenate([hi, lo, hi], axis=-1)


def _pad_dup(X):
    idx = np.concatenate([np.arange(len(X)), np.arange(NPAD - len(X))])
    return X[idx], idx


def _kd_order(X, depth):
    """Balanced kd ordering: permutation putting X into 2^depth equal leaves."""
    n = len(X)
    perm = np.arange(n)[None, :]           # [nseg, seglen]
    for _ in range(depth):
        seg = X[perm]                      # [nseg, seglen, 3]
        ax = np.argmax(seg.max(1) - seg.min(1), axis=1)        # [nseg]
        vals = np.take_along_axis(seg, ax[:, None, None], 2)[:, :, 0]
        order = np.argsort(vals, axis=1, kind="stable")
        perm = np.take_along_axis(perm, order, 1)
        perm = perm.reshape(perm.shape[0] * 2, perm.shape[1] // 2)
    return perm.reshape(-1)


def _select_blocks(Q, X):
    """IVF selection for one direction of one class.

    Q: [nq, 3] queries, X: [nx, 3] candidates (nq, nx >= 1).
    Returns (qord [NPAD], stat15 [QBLK,128,15], mov15 [QBLK,CAND,15])."""
    Qp, _ = _pad_dup(Q)
    Xp, _ = _pad_dup(X)
    qord = _kd_order(Qp, 6)
    xord = _kd_order(Xp, LDEPTH)
    Xo = Xp[xord].reshape(-1, LSZ, 3)                  # [NL, LSZ, 3]
    cent = Xo.mean(1)
    dif = Xo - cent[:, None, :]
    dist_c = np.sqrt((dif * dif).sum(2))
    rad = dist_c.max(1)
    rep = Xo[np.arange(len(Xo)), dist_c.argmin(1)]

    Qs = Qp[qord]
    q2 = (Qs * Qs).sum(1)[:, None]
    dc = np.sqrt(np.maximum(q2 + (cent * cent).sum(1)[None, :] - 2.0 * Qs @ cent.T, 0))
    drep = np.sqrt(np.maximum(q2 + (rep * rep).sum(1)[None, :] - 2.0 * Qs @ rep.T, 0))
    ub = drep.min(1)
    score = (dc - rad[None, :]) - ub[:, None]
    score_b = score.reshape(QBLK, 128, -1).min(1)      # [QBLK, NL]
    take = np.argpartition(score_b, LEAVES_TAKE - 1, axis=1)[:, :LEAVES_TAKE]

    cand = Xo[take].reshape(QBLK, CAND, 3)             # [QBLK, CAND, 3]
    stat15 = _comp15(_aug5(Qs.reshape(QBLK, 128, 3), "stat"), True)
    mov15 = _comp15(_aug5(cand, "mov"), False)
    return qord, stat15, mov15


def kernel(pred_rot_matrix, pred_trans, target_rot_matrix, target_trans,
           model_points, fg_mask, class_ids):
    global _CACHED_NC
    predR = np.asarray(pred_rot_matrix, np.float32).reshape(I, 3, 3)
    predt = np.asarray(pred_trans, np.float32).reshape(I, 3)
    gtR = np.asarray(target_rot_matrix, np.float32).reshape(I, 3, 3)
    gtt = np.asarray(target_trans, np.float32).reshape(I, 3)
    pts = np.asarray(model_points, np.float32)  # [C, P, 3]
    fg = np.asarray(fg_mask).reshape(I).astype(bool)
    cls = np.asarray(class_ids).reshape(I).astype(np.int64)

    in_maps = []
    meta = []
    for c in range(C):
        m = fg & (cls == c)
        idx = np.argsort(~m, kind="stable")[:M]
        valid = m[idx]
        k = int(valid.sum())
        if k == 0:
            meta.append(None)
            in_maps.append({
                "stat": np.zeros((15, NBD, 128), ml_dtypes.bfloat16),
                "mov": np.zeros((15, NBD, CAND), ml_dtypes.bfloat16),
            })
            continue
        pp = np.concatenate(
            [pts[cls[i]] @ predR[i].T + predt[i] for i in idx[:k]], 0
        ).astype(np.float32)
        gp = np.concatenate(
            [pts[cls[i]] @ gtR[i].T + gtt[i] for i in idx[:k]], 0
        ).astype(np.float32)
        qord0, stat0, mov0 = _select_blocks(pp, gp)
        qord1, stat1, mov1 = _select_blocks(gp, pp)
        # stat: [15, NBD, 128]; mov: [15, NBD, CAND]
        stat = np.concatenate([stat0, stat1], 0).transpose(2, 0, 1)
        mov = np.concatenate([mov0, mov1], 0).transpose(2, 0, 1)
        meta.append((k, qord0, qord1))
        in_maps.append({"stat": np.ascontiguousarray(stat),
                        "mov": np.ascontiguousarray(mov)})

    if _CACHED_NC is None:
        _CACHED_NC = _build_graph()
    res = run_bass_kernel_spmd(_CACHED_NC, in_maps, core_ids=list(range(8)))

    total = np.float32(0.0)
    for c in range(C):
        if meta[c] is None:
            continue
        k, qord0, qord1 = meta[c]
        rm = np.asarray(res.results[c]["rowmin"], np.float32).reshape(128, NBD)
        d_acc = np.zeros(k, np.float64)
        for d, qord in ((0, qord0), (1, qord1)):
            vals = rm[:, d * QBLK : (d + 1) * QBLK].T.reshape(-1)  # sorted order
            dmin = np.empty(NPAD, np.float32)
            dmin[qord] = vals
            dd = np.sqrt(np.maximum(dmin[: k * P], EPS))
            d_acc += dd.reshape(k, P).mean(1)
        total += np.float32((0.5 * d_acc).sum())

    n_fg = int(fg.sum())
    if n_fg > 0:
        out = np.float32(total / np.float32(max(n_fg, 1)))
    else:
        out = np.float32(0.0)
    return np.asarray(out, dtype=np.float32)


# revision 19
# speedup vs baseline: 10.7582x; 1.0801x over previous
"""ADDS loss kernel for Trainium2, 8 NeuronCores (SPMD, one class per core).

Math: for each class c, gather its (<=8) valid instances, transform the
class model points by pred/gt poses -> pp/gp point sets.  The loss needs,
for every point, the distance to the nearest point of the other set
(both directions), then sqrt/means/masked sum.

Instead of the full 8192x8192 pairwise matrix (vector-engine bound at
~600us), the host runs an IVF-style coarse selection: candidates are
kd-split into 2^LDEPTH balanced leaves, queries into 64 blocks of 128.
Per query block the host ranks leaves by the exact-safe bound
min_q(lowerbound(q,leaf) - upperbound(q)) and keeps the top LEAVES_TAKE
leaves (CAND candidate points).  The device then computes the exact
d2 = |p|^2+|g|^2-2p.g for the 128x CAND matrix of every block (K=15
compensated bf16 matmul, ~2^-17 accurate) and min-reduces each row:

  - per 4-block group: 4 matmuls fill 4 PSUM banks.
  - first touch of PSUM is split between ScalarE (bf16 drain, then a
    VectorE pair-min) and VectorE (direct pair-min of the two PSUM
    halves) to balance the two engines.
  - a bf16 halving tree reduces to 32 wide; a deferred batched tree
    finishes 32 -> 1 across all 128 block-dirs at once.

Host finishes with sqrt/means/masked sum (identical to reference tail).
"""

import numpy as np
import ml_dtypes

import concourse.tile as tile
from concourse import bacc, mybir
from concourse.bass_utils import run_bass_kernel_spmd

F32 = mybir.dt.float32
BF16 = mybir.dt.bfloat16
AL = mybir.AluOpType

# Problem constants (hardcoded per harness contract)
B, N, C, P = 1, 64, 8, 1000
I = B * N            # 64 instances
M = I // C           # 8 instances per class (static cap, as in reference)
NPAD = 8192          # padded point count per side per class
QBLK = 64            # query blocks of 128 per direction
NBD = 2 * QBLK       # block-dirs per core (2 directions)
NGRP = NBD // 4      # device groups of 4 block-dirs
EPS = 1e-12

# IVF selection config
LDEPTH = 11                      # 2^11 = 2048 leaves of 4 points
LSZ = NPAD >> LDEPTH             # leaf size (4)
CAND = 256                       # candidate points per query block
LEAVES_TAKE = CAND // LSZ        # leaves kept per block
# per-group first-touch schedule: True = split drain (VectorE reads PSUM
# for one half), False = ScalarE drains all of PSUM. 3/8 split fraction
# balances the engines given VectorE also owns the final trees.
SPLIT_PATTERN = (True, False, False, True, False, False, True, False)

_CACHED_NC = None


def _build_graph():
    nc = bacc.Bacc()
    stat_d = nc.declare_dram_parameter("stat", [15, NBD, 128], BF16, isOutput=False)
    mov_d = nc.declare_dram_parameter("mov", [15, NBD, CAND], BF16, isOutput=False)
    rowmin_d = nc.declare_dram_parameter("rowmin", [128, NBD, 1], F32, isOutput=True)

    HC = CAND // 2

    with tile.TileContext(nc) as tc:
        with (
            tc.tile_pool(name="consts", bufs=1) as consts,
            tc.tile_pool(name="psum", bufs=3, space="PSUM") as psum_pool,
            tc.tile_pool(name="mov", bufs=6) as mov_pool,
            tc.tile_pool(name="grp", bufs=3) as grp_pool,
            tc.tile_pool(name="fin", bufs=1) as fin_pool,
        ):
            stat_sb = consts.tile([15, NBD, 128], BF16)
            nc.sync.dma_start(stat_sb[:, 0:16, :], stat_d[:, 0:16, :])

            collect = consts.tile([128, NBD, HC], BF16)
            rowmins = consts.tile([128, NBD, 1], F32)

            QTR = NBD // 4

            def final_tree(lo, hi):
                """Reduce collect[:, lo:hi, :HC] -> rowmins[:, lo:hi] + DMA."""
                nbd = hi - lo
                cur = collect[:, lo:hi, :]
                w = HC
                while w > 1:
                    h = w // 2
                    if h == 1:
                        nxt = rowmins[:, lo:hi, :]
                    else:
                        nxt = fin_pool.tile([128, QTR, h], BF16, tag=f"f{h}")
                        nxt = nxt[:, 0:nbd, :]
                    nc.vector.tensor_tensor(
                        nxt[:], cur[:, :, 0:h], cur[:, :, h:w], AL.min
                    )
                    cur = nxt
                    w = h
                nc.sync.dma_start(rowmin_d[:, lo:hi, :], rowmins[:, lo:hi, :])

            for g in range(NGRP):
                mt = mov_pool.tile([15, 4, CAND], BF16, tag="mt")
                nc.sync.dma_start(mt[:], mov_d[:, 4 * g : 4 * g + 4, :])
                pt = psum_pool.tile([128, 4, CAND], F32, tag="pt")
                for j in range(4):
                    nc.tensor.matmul(
                        pt[:, j, :],
                        lhsT=stat_sb[:, 4 * g + j, :],
                        rhs=mt[:, j, :],
                        start=True,
                        stop=True,
                    )
                out = collect[:, 4 * g : 4 * g + 4, :]
                if SPLIT_PATTERN[g % len(SPLIT_PATTERN)]:
                    # type-B: ScalarE drains one half; VectorE fuses the
                    # pair-min into its own PSUM drain of the other half
                    sc = grp_pool.tile([128, 4, HC], BF16, tag="sch")
                    nc.scalar.copy(sc[:], pt[:, :, 0:HC])
                    nc.vector.tensor_tensor(
                        out[:], pt[:, :, HC:CAND], sc[:], AL.min
                    )
                else:
                    # type-A: ScalarE drains PSUM -> bf16; VectorE pair-min 2x
                    sc = grp_pool.tile([128, 4, CAND], BF16, tag="sc")
                    nc.scalar.copy(sc[:], pt[:])
                    nc.vector.tensor_tensor(
                        out[:], sc[:, :, 0:HC], sc[:, :, HC:CAND], AL.min
                    )
                if g == 0:
                    nc.sync.dma_start(stat_sb[:, 16:48, :], stat_d[:, 16:48, :])
                elif g == 2:
                    nc.sync.dma_start(stat_sb[:, 48:, :], stat_d[:, 48:, :])
                elif g % (NGRP // 4) == NGRP // 4 - 1 and g != NGRP - 1:
                    q = g // (NGRP // 4)
                    final_tree(q * QTR, (q + 1) * QTR)
            final_tree(3 * QTR, NBD)
    nc.compile()
    return nc


def _split_bf16(x):
    """Return (hi, lo) bf16 arrays with hi + lo ~= x (f32)."""
    x = x.astype(np.float32)
    hi = x.astype(ml_dtypes.bfloat16)
    lo = (x - hi.astype(np.float32)).astype(ml_dtypes.bfloat16)
    return hi, lo


def _aug5(pts, side):
    """pts [..., 3] -> aug [..., 5] rows (x,y,z,1,p2) or (-2x,-2y,-2z,g2,1)."""
    sq = (pts.astype(np.float32) ** 2).sum(-1)
    out = np.empty(pts.shape[:-1] + (5,), np.float32)
    if side == "stat":
        out[..., 0:3] = pts
        out[..., 3] = 1.0
        out[..., 4] = sq
    else:
        out[..., 0:3] = -2.0 * pts
        out[..., 3] = sq
        out[..., 4] = 1.0
    return out


def _comp15(aug, stationary):
    """aug [..., 5] f32 -> compensated bf16 [..., 15] (3-term product split)."""
    hi, lo = _split_bf16(aug)
    if stationary:
        return np.concatenate([hi, hi, lo], axis=-1)
    return np.concatenate([hi, lo, hi], axis=-1)


def _pad_dup(X):
    idx = np.concatenate([np.arange(len(X)), np.arange(NPAD - len(X))])
    return X[idx], idx


def _kd_order(X, depth):
    """Balanced kd ordering: permutation putting X into 2^depth equal leaves."""
    n = len(X)
    perm = np.arange(n)[None, :]           # [nseg, seglen]
    for _ in range(depth):
        seg = X[perm]                      # [nseg, seglen, 3]
        ax = np.argmax(seg.max(1) - seg.min(1), axis=1)        # [nseg]
        vals = np.take_along_axis(seg, ax[:, None, None], 2)[:, :, 0]
        order = np.argsort(vals, axis=1, kind="stable")
        perm = np.take_along_axis(perm, order, 1)
        perm = perm.reshape(perm.shape[0] * 2, perm.shape[1] // 2)
    return perm.reshape(-1)


def _select_blocks(Q, X):
    """IVF selection for one direction of one class.

    Q: [nq, 3] queries, X: [nx, 3] candidates (nq, nx >= 1).
    Returns (qord [NPAD], stat15 [QBLK,128,15], mov15 [QBLK,CAND,15])."""
    Qp, _ = _pad_dup(Q)
    Xp, _ = _pad_dup(X)
    qord = _kd_order(Qp, 6)
    xord = _kd_order(Xp, LDEPTH)
    Xo = Xp[xord].reshape(-1, LSZ, 3)                  # [NL, LSZ, 3]
    cent = Xo.mean(1)
    dif = Xo - cent[:, None, :]
    dist_c = np.sqrt((dif * dif).sum(2))
    rad = dist_c.max(1)
    rep = Xo[np.arange(len(Xo)), dist_c.argmin(1)]

    Qs = Qp[qord]
    q2 = (Qs * Qs).sum(1)[:, None]
    dc = np.sqrt(np.maximum(q2 + (cent * cent).sum(1)[None, :] - 2.0 * Qs @ cent.T, 0))
    drep = np.sqrt(np.maximum(q2 + (rep * rep).sum(1)[None, :] - 2.0 * Qs @ rep.T, 0))
    ub = drep.min(1)
    score = (dc - rad[None, :]) - ub[:, None]
    score_b = score.reshape(QBLK, 128, -1).min(1)      # [QBLK, NL]
    take = np.argpartition(score_b, LEAVES_TAKE - 1, axis=1)[:, :LEAVES_TAKE]

    cand = Xo[take].reshape(QBLK, CAND, 3)             # [QBLK, CAND, 3]
    stat15 = _comp15(_aug5(Qs.reshape(QBLK, 128, 3), "stat"), True)
    mov15 = _comp15(_aug5(cand, "mov"), False)
    return qord, stat15, mov15


def kernel(pred_rot_matrix, pred_trans, target_rot_matrix, target_trans,
           model_points, fg_mask, class_ids):
    global _CACHED_NC
    predR = np.asarray(pred_rot_matrix, np.float32).reshape(I, 3, 3)
    predt = np.asarray(pred_trans, np.float32).reshape(I, 3)
    gtR = np.asarray(target_rot_matrix, np.float32).reshape(I, 3, 3)
    gtt = np.asarray(target_trans, np.float32).reshape(I, 3)
    pts = np.asarray(model_points, np.float32)  # [C, P, 3]
    fg = np.asarray(fg_mask).reshape(I).astype(bool)
    cls = np.asarray(class_ids).reshape(I).astype(np.int64)

    in_maps = []
    meta = []
    for c in range(C):
        m = fg & (cls == c)
        idx = np.argsort(~m, kind="stable")[:M]
        valid = m[idx]
        k = int(valid.sum())
        if k == 0:
            meta.append(None)
            in_maps.append({
                "stat": np.zeros((15, NBD, 128), ml_dtypes.bfloat16),
                "mov": np.zeros((15, NBD, CAND), ml_dtypes.bfloat16),
            })
            continue
        pp = np.concatenate(
            [pts[cls[i]] @ predR[i].T + predt[i] for i in idx[:k]], 0
        ).astype(np.float32)
        gp = np.concatenate(
            [pts[cls[i]] @ gtR[i].T + gtt[i] for i in idx[:k]], 0
        ).astype(np.float32)
        qord0, stat0, mov0 = _select_blocks(pp, gp)
        qord1, stat1, mov1 = _select_blocks(gp, pp)
        # stat: [15, NBD, 128]; mov: [15, NBD, CAND]
        stat = np.concatenate([stat0, stat1], 0).transpose(2, 0, 1)
        mov = np.concatenate([mov0, mov1], 0).transpose(2, 0, 1)
        meta.append((k, qord0, qord1))
        in_maps.append({"stat": np.ascontiguousarray(stat),
                        "mov": np.ascontiguousarray(mov)})

    if _CACHED_NC is None:
        _CACHED_NC = _build_graph()
    res = run_bass_kernel_spmd(_CACHED_NC, in_maps, core_ids=list(range(8)))

    total = np.float32(0.0)
    for c in range(C):
        if meta[c] is None:
            continue
        k, qord0, qord1 = meta[c]
        rm = np.asarray(res.results[c]["rowmin"], np.float32).reshape(128, NBD)
        d_acc = np.zeros(k, np.float64)
        for d, qord in ((0, qord0), (1, qord1)):
            vals = rm[:, d * QBLK : (d + 1) * QBLK].T.reshape(-1)  # sorted order
            dmin = np.empty(NPAD, np.float32)
            dmin[qord] = vals
            dd = np.sqrt(np.maximum(dmin[: k * P], EPS))
            d_acc += dd.reshape(k, P).mean(1)
        total += np.float32((0.5 * d_acc).sum())

    n_fg = int(fg.sum())
    if n_fg > 0:
        out = np.float32(total / np.float32(max(n_fg, 1)))
    else:
        out = np.float32(0.0)
    return np.asarray(out, dtype=np.float32)
